# revision 27
# baseline (speedup 1.0000x reference)
"""DMPNN encoder on 8 TRN2 cores via Bass/Tile — v8 (fp8 + DoubleRow + split
collective). ~1.28 ms (v5) -> ~0.96 ms.

Key points vs v5:
  * fp8(e4m3) end to end on the message path: imsg tables (PTf), one-hot
    scatter/gather matrices (S, GT), messages, B tables and gathered rows
    (512 B strided rows -- dma_gather needs 256 B elem granularity). Host sim
    puts rel err at ~2.4e-3 vs the 2e-2 gate; measured 4.5e-3. Wh/Wo/readout
    stay bf16/f32.
  * sweep1 reads a host-relu'd msg1 table (PTf_msg1): no on-device relu at all
    in sweep1.
  * fp8 DoubleRow matmuls for all three scatter passes: one PE instruction
    accumulates two 128-token chunks (lhsT [128,2,128] S-pair, rhs [128,2,320]
    msg-pair; out += sum_i S_i^T @ msg_i). The S tables and 2-chunk msg
    batches already have this exact layout, so only the matmul calls changed.
  * B2 AllGather split in two (tiles 0..TS-1 / TS..39): collective A fires
    ~25% into sweep1 so the descriptor-bound dma_gathers (~430 us of gpsimd
    Q7 time -- the critical resource) start while sweep1 still runs;
    collective B's ~200 us (CC cores run collectives on the slow iDMA path)
    hides under the gather-A stream. Gather buckets are (dst-tile<TS,
    src tile); int16 gather indices address the A/B half tables separately
    (both < 32768 rows -- this bounds TS to [8, 31]).
  * Readout paired (2 tiles per Wo matmul set, 256-wide moving) and emitted
    inline in the sweep3 loop. NOTE: a PSUM accumulation that spans two banks
    needs start=True per bank -- start only clears has_written in the bank it
    writes (this was a silent-wrong-answer bug).
  * Chip-level clock throttling causes ~10% run-to-run wall-time variance;
    compare kernels by trace engine-busy breakdowns, not single wall times.

  sweep1 (src order): msg1 slab (f8) -> DoubleRow scatter -> Wh tail ->
    B2A/B2B f8; collectives A/B -> BfullA/B (Shared).
  sweep2 (src order): dma_gather Bfull[dst] f8 rows; DVE add + Scalar relu;
    DoubleRow scatter -> A3; per-tile Wh tail -> B3 (f8, SBUF resident).
  sweep3 (dst order): agg = GT.T @ B3_tile; add+relu; DoubleRow scatter ->
    in_agg; paired readout per tile; final tiny AllReduce + Wout.
"""
import numpy as np
import ml_dtypes

BF16 = ml_dtypes.bfloat16
F8 = ml_dtypes.float8_e4m3fn

NODE_F = 117
EDGE_F = 10
H = 300
DEPTH = 3
W = 320                           # padded hidden width on the message path
BROW = 512                        # fp8 elements per B-table row (512B, 256B-aligned)
T1 = 11                           # tiles 0..T1-1 = gather section A
T2 = 25                           # tiles T1..T2-1 = section B1; rest = B2


# ---------------------------------------------------------------- host side

def _pack_idx(idx):
    """[TOK] int -> [128, TOK/16] int16 in dma_gather wrap layout."""
    idx = np.asarray(idx, np.int64)
    assert len(idx) % 16 == 0
    a = idx.reshape(-1, 16).T.astype(np.int16)
    assert (idx < 32768).all() and (idx >= 0).all()
    return np.tile(a, (8, 1))


def preprocess(atom, ef, src, dst, Wi, bi, Wh, bh, Wo, bo, Wout, bout, C=8, gblk=20):
    N, E = atom.shape[0], src.shape[0]
    assert N % C == 0
    npc = N // C
    TPB = npc // 128 + 1          # always >= 1 pad row per core
    NPC = TPB * 128
    AROWS = T1 * 128              # per-core rows per gather section
    B1ROWS = (T2 - T1) * 128
    B2ROWS = NPC - T2 * 128       # includes pads
    ZR2 = B2ROWS - 1              # a guaranteed-zero row in core 0's B2 block

    deg_src = np.bincount(src, minlength=N)
    self_loop = src == dst
    has_nb = (deg_src[dst] - self_loop.astype(np.int64)) > 0
    deg_in = np.bincount(dst, minlength=N)

    meta = dict(C=C, N=N, E=E, npc=npc, TPB=TPB, NPC=NPC, AROWS=AROWS,
                B1ROWS=B1ROWS, B2ROWS=B2ROWS, orders={})
    percore = [dict() for _ in range(C)]

    # full input message per edge, f32 on host: P[dst] + ef@WiE.T + bi + bh*has_nb
    P = atom.astype(np.float32) @ Wi[:, :NODE_F].T.astype(np.float32)      # [N, 300]
    imsg = P[dst] + ef.astype(np.float32) @ Wi[:, NODE_F:].T.astype(np.float32)
    imsg += bi[None, :] + bh[None, :] * has_nb[:, None].astype(np.float32)  # [E, 300]

    for c in range(C):
        lo = c * npc
        # masked transposed readout table with mask row 127 (bakes bo + deg_in mask)
        atr = np.zeros((128, NPC), BF16)
        msk = (deg_in[lo:lo + npc] > 0)
        atr[:NODE_F, :npc] = (atom[lo:lo + npc].T * msk[None, :]).astype(BF16)
        atr[127, :npc] = msk.astype(BF16)
        percore[c]["atomT_read"] = atr

    # ---- weights (shared, replicated)
    shared = {}
    wht = np.zeros((320, W), BF16)
    wht[:H, :H] = Wh.T.astype(BF16)
    shared["wht0"] = wht[0:128]
    shared["wht1"] = wht[128:256]
    shared["wht2"] = wht[256:320]
    wo = np.zeros((448, 384), BF16)   # K rows: 0..127 atom(+mask@127), 128.. in_agg
    wo[:NODE_F, :H] = Wo[:, :NODE_F].T.astype(BF16)
    wo[127, :H] = bo.astype(BF16)
    wo[128:128 + H, :H] = Wo[:, NODE_F:].T.astype(BF16)
    shared["wo_ka"] = wo[0:128]
    shared["wo_k2"] = wo[128:256]
    shared["wo_k3"] = wo[256:384]
    shared["wo_k4"] = wo[384:448]
    wout = np.zeros((384, 320), np.float32)
    wout[:H, :H] = (Wout.T / N).astype(np.float32)
    shared["woutt0"] = wout[0:128]
    shared["woutt1"] = wout[128:256]
    shared["woutt2"] = wout[256:384]
    shared["bout_row"] = np.pad(bout.astype(np.float32), (0, 20))[None, :]
    shared["one_t"] = np.ones((1, 1), np.float32)
    shared["ident"] = np.eye(128, dtype=BF16)

    downer = dst // npc
    dloc = dst - downer * npc
    dtile = dloc // 128

    # ---- per-order token layouts
    for order in ("src", "dst"):
        key = src if order == "src" else dst
        owner = key // npc
        loc = key - owner * npc
        tile_of = loc // 128
        halves = 3 if order == "src" else 1
        if order == "src":
            half_of = np.where(has_nb & (dtile < T1), 0,
                               np.where(has_nb & (dtile < T2), 1, 2))
        else:
            half_of = np.zeros(E, np.int64)

        # vectorized per-(core, h, t) bucketing
        gid = (owner * halves + half_of) * TPB + tile_of
        counts = np.bincount(gid, minlength=C * halves * TPB).reshape(C, halves, TPB)
        n_chunks = -(-counts.max(axis=0) // 128)  # [halves, TPB]
        # copy-on-first-visit flushes need every tile visited at least once
        assert (n_chunks.sum(axis=0) > 0).all()
        blk0 = np.zeros((halves, TPB), np.int64)
        acc = 0
        for h in range(halves):
            for t in range(TPB):
                blk0[h, t] = acc
                acc += n_chunks[h, t]
        TOTBLK = int(acc)
        TOK = TOTBLK * 128

        # slab groups: contiguous tiles within a half, ~gblk chunks each
        groups = []
        for h in range(halves):
            t = 0
            while t < TPB:
                t0, nb = t, 0
                while t < TPB and (nb == 0 or nb + n_chunks[h, t] <= gblk):
                    nb += n_chunks[h, t]
                    t += 1
                if nb:
                    groups.append(dict(h=h, t0=t0, t1=t, b0=int(blk0[h, t0]),
                                       b1=int(blk0[h, t - 1] + n_chunks[h, t - 1])))
        om = dict(halves=halves, n_chunks=n_chunks, blk0=blk0, TOTBLK=TOTBLK,
                  TOK=TOK, groups=groups)
        meta["orders"][order] = om

        # vectorized token assignment: stable-sort edges by gid, position within
        # group + per-(h,t) chunk base gives each edge its token slot
        ordr = np.argsort(gid, kind="stable")
        sorted_gid = gid[ordr]
        grp_starts = np.searchsorted(sorted_gid, np.arange(C * halves * TPB))
        within = np.arange(E) - grp_starts[sorted_gid]
        base_tok = np.broadcast_to((blk0 * 128)[None], (C, halves, TPB)).reshape(-1)
        tok_sorted = base_tok[sorted_gid] + within
        tok = np.empty(E, np.int64)
        tok[ordr] = tok_sorted

        if order == "src":
            vA = downer * AROWS + dloc
            vB1 = downer * B1ROWS + (dloc - T1 * 128)
            vB2 = downer * B2ROWS + (dloc - T2 * 128)
            v = np.where(half_of == 0, vA,
                         np.where(half_of == 1, vB1,
                                  np.where(has_nb, vB2, ZR2)))

        for c in range(C):
            sel = owner == c
            tk = tok[sel]
            # PTf: [128, TOTBLK, W] f8, PTf[p, b, :] = imsg[token b*128+p]
            ptf = np.zeros((TOTBLK * 128, W), F8)
            ptf[tk, :H] = imsg[sel].astype(F8)
            percore[c][f"PTf_{order}"] = \
                ptf.reshape(TOTBLK, 128, W).transpose(1, 0, 2).copy()
            if order == "src":
                # sweep1 reads host-relu'd msg1 in fp8: no on-device relu
                m1 = np.zeros((TOTBLK * 128, W), F8)
                m1[tk, :H] = np.maximum(imsg[sel], 0.0).astype(F8)
                percore[c]["PTf_msg1"] = \
                    m1.reshape(TOTBLK, 128, W).transpose(1, 0, 2).copy()
            S = np.zeros((128, TOTBLK, 128), F8)
            S[tk % 128, tk // 128, (loc[sel] - tile_of[sel] * 128)] = 1.0
            percore[c][f"S_{order}"] = S
            if order == "src":
                # pad tokens gather row 0 of their section (value ignored)
                idxB = np.zeros(TOK, np.int64)
                idxB[tk] = v[sel]
                percore[c]["idxB_src"] = _pack_idx(idxB)
            else:
                # GT: one-hot [node_in_tile, blk, tok_in_chunk] for matmul-gather
                GT = np.zeros((128, TOTBLK, 128), F8)
                GT[(loc[sel] - tile_of[sel] * 128), tk // 128, tk % 128] = 1.0
                percore[c]["GT_dst"] = GT

    in_maps = []
    for c in range(C):
        m = dict(shared)
        m.update(percore[c])
        in_maps.append(m)
    return meta, in_maps


# ---------------------------------------------------------------- device side

def build_nc(meta, debug=False):
    import concourse.bass as bass
    import concourse.tile as tile
    from concourse import bacc, mybir
    from concourse.library_config import mlp

    C, NPC, TPB = meta["C"], meta["NPC"], meta["TPB"]
    AROWS, npc = meta["AROWS"], meta["npc"]
    B1ROWS, B2ROWS = meta["B1ROWS"], meta["B2ROWS"]
    f32, bf16, i16 = mybir.dt.float32, mybir.dt.bfloat16, mybir.dt.int16
    f8 = mybir.dt.float8e4
    ADD = mybir.AluOpType.add
    MAX = mybir.AluOpType.max

    nc = bacc.Bacc("TRN2", target_bir_lowering=False, debug=debug, num_devices=C)

    def din(name, shape, dt):
        return nc.dram_tensor(name, shape, dt, kind="ExternalInput")

    oms = meta["orders"]
    atomT_read = din("atomT_read", [128, NPC], bf16)
    ins = {}
    for o in ("src", "dst"):
        om = oms[o]
        ins[f"PTf_{o}"] = din(f"PTf_{o}", [128, om["TOTBLK"], W], f8)
        ins[f"S_{o}"] = din(f"S_{o}", [128, om["TOTBLK"], 128], f8)
    ins["PTf_msg1"] = din("PTf_msg1", [128, oms["src"]["TOTBLK"], W], f8)
    ins["idxB_src"] = din("idxB_src", [128, oms["src"]["TOK"] // 16], i16)
    ins["GT_dst"] = din("GT_dst", [128, oms["dst"]["TOTBLK"], 128], f8)
    wht = [din(f"wht{i}", [128 if i < 2 else 64, W], bf16) for i in range(3)]
    wo_ka = din("wo_ka", [128, 384], bf16)
    wo_k2 = din("wo_k2", [128, 384], bf16)
    wo_k3 = din("wo_k3", [128, 384], bf16)
    wo_k4 = din("wo_k4", [64, 384], bf16)
    woutt = [din(f"woutt{i}", [128, 320], f32) for i in range(3)]
    bout_row = din("bout_row", [1, 320], f32)
    one_t = din("one_t", [1, 1], f32)
    ident = din("ident", [128, 128], bf16)
    out_d = nc.dram_tensor("out", [1, 320], f32, kind="ExternalOutput")

    with tile.TileContext(nc) as tc:
        nc.gpsimd.load_library(mlp)
        import contextlib
        ctx = contextlib.ExitStack()
        with ctx:
            cpool = ctx.enter_context(tc.tile_pool(name="consts", bufs=1))
            idxpool = ctx.enter_context(tc.tile_pool(name="idx", bufs=1))
            pt8pool = ctx.enter_context(tc.tile_pool(name="PTf8", bufs=3))
            ptpool = ctx.enter_context(tc.tile_pool(name="PTf", bufs=3))
            sdpool = ctx.enter_context(tc.tile_pool(name="Sdst", bufs=3))
            gtpool = ctx.enter_context(tc.tile_pool(name="GT", bufs=2))
            gpool = ctx.enter_context(tc.tile_pool(name="gB", bufs=4))
            mpool = ctx.enter_context(tc.tile_pool(name="msg", bufs=4))
            accpool = ctx.enter_context(tc.tile_pool(name="acc", bufs=2))
            b3pool = ctx.enter_context(tc.tile_pool(name="B3", bufs=1))
            trpool = ctx.enter_context(tc.tile_pool(name="tr", bufs=3))
            smallpool = ctx.enter_context(tc.tile_pool(name="small", bufs=4))
            ps_big = ctx.enter_context(tc.tile_pool(name="ps_big", bufs=3, space="PSUM"))
            ps_bigr = ctx.enter_context(tc.tile_pool(name="ps_bigr", bufs=1, space="PSUM"))
            ps_at = ctx.enter_context(tc.tile_pool(name="ps_at", bufs=2, space="PSUM"))
            ps_tr = ctx.enter_context(tc.tile_pool(name="ps_tr", bufs=1, space="PSUM"))
            dram = ctx.enter_context(tc.tile_pool(name="dram", bufs=1, space="DRAM"))

            def cload(t, shape, dt):
                s = cpool.tile(shape, dt, tag=t.name)
                nc.sync.dma_start(s[:], t[:])
                return s

            wht_s = [cload(w, [128 if i < 2 else 64, W], bf16) for i, w in enumerate(wht)]
            wo_ka_s = cload(wo_ka, [128, 384], bf16)
            wo_k2_s = cload(wo_k2, [128, 384], bf16)
            wo_k3_s = cload(wo_k3, [128, 384], bf16)
            wo_k4_s = cload(wo_k4, [64, 384], bf16)
            woutt_s = [cload(w, [128, 320], f32) for w in woutt]
            bout_s = cload(bout_row, [1, 320], f32)
            one_s = cload(one_t, [1, 1], f32)
            ident_s = cload(ident, [128, 128], bf16)

            idxB = idxpool.tile([128, oms["src"]["TOK"] // 16], i16, tag="idxB")
            nc.sync.dma_start(idxB[:], ins["idxB_src"][:])

            B2A = dram.tile([AROWS, BROW], f8)
            B2B1 = dram.tile([B1ROWS, BROW], f8)
            B2B2 = dram.tile([B2ROWS, BROW], f8)
            BfullA = dram.tile([C * AROWS, BROW], f8, addr_space="Shared")
            BfullB1 = dram.tile([C * B1ROWS, BROW], f8, addr_space="Shared")
            BfullB2 = dram.tile([C * B2ROWS, BROW], f8, addr_space="Shared")

            def a_transpose(acc_t, m):
                """A^T h-block m: PE transpose + Scalar PSUM->SBUF copy."""
                hi = 128 if m < 2 else 64
                tp = ps_tr.tile([128, 128], bf16, tag="tr")
                nc.tensor.transpose(
                    tp[0:hi, :], acc_t[:, m * 128: m * 128 + hi], ident_s[:])
                sb = trpool.tile([128, 128], bf16, tag=f"tr{m}")
                nc.scalar.copy(sb[0:hi, :], tp[0:hi, :])
                return sb

            # per-(sweep, tile) state
            acc_of = [{}, {}, {}]      # k-1 -> t -> acc tile
            b3_of = {}                 # t -> B3 tile [128, W] f8

            def emit_group(k, g):
                order = "src" if k < 3 else "dst"
                om = oms[order]
                h, b0, b1 = g["h"], g["b0"], g["b1"]
                nb = b1 - b0
                ntok = nb * 128
                first_half = [0 if om["n_chunks"][0][t] > 0 else 1
                              for t in range(TPB)]
                if k == 1:
                    ptf = pt8pool.tile([128, nb, W], f8, tag="ptf8")
                    nc.sync.dma_start(ptf[:], ins["PTf_msg1"][:, b0:b1, :])
                else:
                    ptf = ptpool.tile([128, nb, W], f8, tag="ptf")
                    nc.sync.dma_start(ptf[:], ins[f"PTf_{order}"][:, b0:b1, :])
                sd = sdpool.tile([128, nb, 128], f8, tag="Sd")
                nc.sync.dma_start(sd[:], ins[f"S_{order}"][:, b0:b1, :])
                Ssb = sd[:]
                gB = None
                GTsb = None
                if k == 2:
                    gB = gpool.tile([128, nb, BROW], f8, tag="gB")
                    tab = [BfullA, BfullB1, BfullB2][h]
                    nc.gpsimd.dma_gather(
                        gB[:], tab[:, :],
                        idxB[:, b0 * 8:b0 * 8 + ntok // 16],
                        ntok, ntok, BROW, single_packet=False)
                if k == 3:
                    GTsb = gtpool.tile([128, nb, 128], f8, tag="GT")
                    nc.sync.dma_start(GTsb[:], ins["GT_dst"][:, b0:b1, :])
                for t in range(g["t0"], g["t1"]):
                    nchk = int(om["n_chunks"][h][t])
                    if nchk == 0:
                        continue
                    at_ps = ps_at.tile([128, W], f32, tag="at_ps")
                    jb0 = int(om["blk0"][h][t])
                    jj = 0
                    while jj < nchk:
                        # 2-chunk batches share one msg tile / one Scalar relu
                        nba = min(2, nchk - jj) if k > 1 else 1
                        if k == 1:
                            jr = jb0 + jj - b0
                            if jj + 1 < nchk:
                                nc.tensor.matmul(
                                    at_ps[:], Ssb[:, jr:jr + 2, :],
                                    ptf[:, jr:jr + 2, :],
                                    start=(jj == 0), stop=(jj + 2 == nchk),
                                    perf_mode=mybir.MatmulPerfMode.DoubleRow,
                                    skip_group_check=True)
                                jj += 2
                            else:
                                nc.tensor.matmul(
                                    at_ps[:], Ssb[:, jr, :], ptf[:, jr, :],
                                    start=(jj == 0), stop=True,
                                    skip_group_check=True)
                                jj += 1
                            continue
                        msg = mpool.tile([128, nba, W], f8, tag=f"msg{nba}")
                        jrf = jb0 + jj - b0
                        if k == 2:
                            # one DVE op covers the whole 2-chunk batch
                            nc.vector.tensor_tensor(
                                msg[:, 0:nba, :], ptf[:, jrf:jrf + nba, :],
                                gB[:, jrf:jrf + nba, 0:W], ADD)
                        else:
                            for a in range(nba):
                                jr = jrf + a
                                ag_ps = ps_big.tile([128, W], f32, tag="big")
                                nc.tensor.matmul(
                                    ag_ps[:], GTsb[:, jr, :], b3_of[t][:],
                                    start=True, stop=True,
                                    skip_group_check=True)
                                nc.vector.tensor_tensor(
                                    msg[:, a, :], ag_ps[:], ptf[:, jr, :], ADD)
                        nc.scalar.activation(msg[:], msg[:],
                                             mybir.ActivationFunctionType.Relu)
                        jr0 = jb0 + jj - b0
                        if nba == 2:
                            nc.tensor.matmul(
                                at_ps[:], Ssb[:, jr0:jr0 + 2, :], msg[:, :, :],
                                start=(jj == 0), stop=(jj + 2 == nchk),
                                perf_mode=mybir.MatmulPerfMode.DoubleRow,
                                skip_group_check=True)
                        else:
                            nc.tensor.matmul(
                                at_ps[:], Ssb[:, jr0, :], msg[:, 0, :],
                                start=(jj == 0), stop=(jj + 1 == nchk),
                                skip_group_check=True)
                        jj += nba
                    if h == first_half[t]:
                        acc_t = accpool.tile([128, W], bf16, tag=f"a{t}")
                        acc_of[k - 1][t] = acc_t
                        nc.vector.tensor_copy(acc_t[:], at_ps[:])
                    else:
                        acc_t = acc_of[k - 1][t]
                        nc.vector.tensor_tensor(acc_t[:], at_ps[:], acc_t[:], ADD)

            def emit_tail(k, t):
                """B tile t = A_t @ Wh.T via 3 transposed-block matmuls."""
                b_ps = ps_big.tile([128, W], f32, tag="big")
                for m in range(3):
                    hi = 128 if m < 2 else 64
                    atb = a_transpose(acc_of[k - 1][t], m)
                    nc.tensor.matmul(
                        b_ps[:], atb[0:hi, :], wht_s[m][:],
                        start=(m == 0), stop=(m == 2),
                        skip_group_check=True)
                if k == 1:
                    bsb = mpool.tile([128, W], f8, tag="msg1")
                    nc.scalar.copy(bsb[:], b_ps[:])
                    if t < T1:
                        nc.sync.dma_start(B2A[t * 128:(t + 1) * 128, 0:W], bsb[:])
                    elif t < T2:
                        r0 = (t - T1) * 128
                        nc.sync.dma_start(B2B1[r0:r0 + 128, 0:W], bsb[:])
                    else:
                        r0 = (t - T2) * 128
                        nc.sync.dma_start(B2B2[r0:r0 + 128, 0:W], bsb[:])
                else:
                    b3 = b3pool.tile([128, W], f8, tag=f"b3_{t}")
                    b3_of[t] = b3
                    nc.scalar.copy(b3[:], b_ps[:])

            acc = smallpool.tile([128, 3], f32, tag="acc")
            nc.vector.memset(acc[:], 0.0)

            def emit_readout2(t):
                """Readout for the tile pair (t, t+1): 256-wide moving ops."""
                atr = smallpool.tile([128, 256], bf16, tag="atr")
                nc.sync.dma_start(atr[:], atomT_read[:, t * 128:(t + 2) * 128])
                ia = []
                for m in range(3):
                    hi = 128 if m < 2 else 64
                    sb = trpool.tile([128, 256], bf16, tag=f"rtr{m}")
                    for p in range(2):
                        tp = ps_tr.tile([128, 128], bf16, tag="tr")
                        nc.tensor.transpose(
                            tp[0:hi, :],
                            acc_of[2][t + p][:, m * 128: m * 128 + hi],
                            ident_s[:])
                        nc.scalar.copy(sb[0:hi, p * 128:(p + 1) * 128],
                                       tp[0:hi, :])
                    ia.append(sb)
                ar_ps = ps_bigr.tile([128, 768], f32, tag="bigr")
                for m in range(3):
                    # per-m start/stop: ar_ps spans two PSUM banks and start
                    # only clears has_written for the bank it writes
                    dstp = ar_ps[:, m * 256:(m + 1) * 256]
                    nc.tensor.matmul(dstp, wo_ka_s[:, m * 128:(m + 1) * 128],
                                     atr[:], start=True, stop=False,
                                     skip_group_check=True)
                    nc.tensor.matmul(dstp, wo_k2_s[:, m * 128:(m + 1) * 128],
                                     ia[0][:], start=False, stop=False,
                                     skip_group_check=True)
                    nc.tensor.matmul(dstp, wo_k3_s[:, m * 128:(m + 1) * 128],
                                     ia[1][:], start=False, stop=False,
                                     skip_group_check=True)
                    nc.tensor.matmul(dstp, wo_k4_s[:, m * 128:(m + 1) * 128],
                                     ia[2][0:64, :], start=False,
                                     stop=True, skip_group_check=True)
                arsb = mpool.tile([128, 768], f32, tag="ar")
                nc.vector.tensor_scalar(arsb[:], ar_ps[:], 0.0, None, MAX)
                red = smallpool.tile([128, 3], f32, tag="red")
                for m in range(3):
                    nc.vector.reduce_sum(
                        red[:, m:m + 1], arsb[:, m * 256:(m + 1) * 256],
                        axis=bass.mybir.AxisListType.X)
                nc.vector.tensor_tensor(acc[:], red[:], acc[:], ADD)

            # ---------------- sweep 1 (tails + early collectives A/B1) -----
            g1 = oms["src"]["groups"]
            g1h0 = [g for g in g1 if g["h"] == 0]
            g1h1 = [g for g in g1 if g["h"] == 1]
            g1h2 = [g for g in g1 if g["h"] == 2]
            i0, covered0 = 0, 0
            i1, covered1 = 0, 0
            cA_done = cB1_done = False
            for g in g1h2:
                while covered1 < g["t1"]:
                    gh1 = g1h1[i1]
                    while covered0 < gh1["t1"]:
                        emit_group(1, g1h0[i0])
                        covered0 = g1h0[i0]["t1"]
                        i0 += 1
                    emit_group(1, gh1)
                    covered1 = gh1["t1"]
                    i1 += 1
                while covered0 < g["t1"]:
                    emit_group(1, g1h0[i0])
                    covered0 = g1h0[i0]["t1"]
                    i0 += 1
                emit_group(1, g)
                for t in range(g["t0"], g["t1"]):
                    emit_tail(1, t)
                if g["t1"] >= T1 and not cA_done:
                    nc.gpsimd.collective_compute(
                        "AllGather", bass.mybir.AluOpType.bypass,
                        replica_groups=[list(range(C))],
                        ins=[B2A.opt()], outs=[BfullA.opt()])
                    cA_done = True
                if g["t1"] >= T2 and not cB1_done:
                    nc.gpsimd.collective_compute(
                        "AllGather", bass.mybir.AluOpType.bypass,
                        replica_groups=[list(range(C))],
                        ins=[B2B1.opt()], outs=[BfullB1.opt()])
                    cB1_done = True

            # ---------------- sweeps 2 and 3, interleaved ----------------
            g2 = oms["src"]["groups"]
            g2h0 = [g for g in g2 if g["h"] == 0]
            g2h1 = [g for g in g2 if g["h"] == 1]
            g2h2 = [g for g in g2 if g["h"] == 2]
            pending3 = list(oms["dst"]["groups"])

            for i, g in enumerate(g2h0):
                emit_group(2, g)
                if i == 1:
                    # collective B2 trigger: waits sweep1 tails; CC runs it
                    # under gather-A/B1
                    nc.gpsimd.collective_compute(
                        "AllGather", bass.mybir.AluOpType.bypass,
                        replica_groups=[list(range(C))],
                        ins=[B2B2.opt()], outs=[BfullB2.opt()])
            for g in g2h1:
                emit_group(2, g)
            for g in g2h2:
                emit_group(2, g)
                for t in range(g["t0"], g["t1"]):
                    emit_tail(2, t)
                while pending3 and pending3[0]["t1"] <= g["t1"]:
                    g3 = pending3.pop(0)
                    emit_group(3, g3)
                    for t in range(g3["t0"], g3["t1"]):
                        if t % 2 == 1:
                            emit_readout2(t - 1)
            for g3 in pending3:
                emit_group(3, g3)
                for t in range(g3["t0"], g3["t1"]):
                    if t % 2 == 1:
                        emit_readout2(t - 1)

            # ---------------- final reduce ----------------
            accd = dram.tile([128, 3], f32)
            accr_d = dram.tile([128, 3], f32)
            accsb = smallpool.tile([128, 3], f32, tag="accr")
            nc.sync.dma_start(accd[:], acc[:])
            nc.gpsimd.collective_compute(
                "AllReduce", bass.mybir.AluOpType.add,
                replica_groups=[list(range(C))],
                ins=[accd.opt()], outs=[accr_d.opt()])
            nc.sync.dma_start(accsb[:], accr_d[:])
            o_ps = ps_big.tile([1, 320], f32, tag="big")
            for cc in range(3):
                nc.tensor.matmul(o_ps[:], accsb[:, cc:cc + 1], woutt_s[cc][:],
                                 start=(cc == 0), stop=False,
                                 skip_group_check=True)
            nc.tensor.matmul(o_ps[:], one_s[:], bout_s[:],
                             start=False, stop=True, skip_group_check=True)
            osb = smallpool.tile([1, 320], f32, tag="osb")
            nc.vector.tensor_scalar(osb[:], o_ps[:], 0.0, None, MAX)
            nc.sync.dma_start(out_d[:], osb[:])

    nc.compile()
    return nc


_last_results = None


def kernel(**inputs):
    """Full-shape entry point: returns [300] float32."""
    global _last_results
    trace = bool(inputs.pop("_trace", False))
    atom = np.asarray(inputs["atom_features"], np.float32)
    ef = np.asarray(inputs["edge_features"], np.float32)
    src = np.asarray(inputs["edge_src"]).astype(np.int64)
    dst = np.asarray(inputs["edge_dst"]).astype(np.int64)
    args = [atom, ef, src, dst] + [np.asarray(inputs[k], np.float32) for k in
                                   ("Wi", "bi", "Wh", "bh", "Wo", "bo", "Wout", "bout")]
    meta, in_maps = preprocess(*args)
    nc = build_nc(meta)
    from concourse.bass_utils import run_bass_kernel_spmd
    res = run_bass_kernel_spmd(nc, in_maps, list(range(meta["C"])), trace=trace)
    _last_results = res
    out = np.asarray(res.results[0]["out"]).reshape(-1)[:H].astype(np.float32)
    return out
